# revision 41
# baseline (speedup 1.0000x reference)
"""Trainium2 Bass kernel for nn_MistralAttentionQ (B=1, S=2048, H=4096, 32 Q heads,
8 KV heads, D=128), tensor-parallel over heads across 8 NeuronCores.

Per core c: 4 Q heads (rows 512c:512c+512 of wq) + 1 KV head (rows 128c:128c+128
of wk/wv).  Everything is kept in transposed [feature, seq] layout so that no
on-device activation transposes are needed:

  - Qt/Kt projections:  out[d_tile, s_chunk] = wqkv_t_tile.T @ xT_chunk
  - RoPE rotate_half:   rot(Q) = Prot @ Qt  (tiny 128x128 matmul), then
                        q' = q*cosT + rot(q)*sinT on VectorE
  - V projection:       V[s_tile, d] = xT_tile.T @ wv_t  ([S, D] layout)
  - scores (transposed): sT[k_tile, q_chunk] = Kt_tile.T @ Qt_chunk
  - softmax: exp on ScalarE (scale folded in, no max subtraction -- scores are
    bounded ~ +-10 for this problem); row sums via an all-ones matmul on the
    TensorEngine (partition-dim reduce); masking is multiplicative exp(mask)
    applied only to "mixed" blocks; fully-masked blocks are skipped.
  - PV:                 attn_t[d, q_chunk] += V_ktile.T @ expT[k_tile, q_chunk]
  - AllGather attn_t shards -> [4096, 2048], then o_proj slice:
                        out[s_tile, 512c:512c+512] = attn_all_tile.T @ wo_t
All matmuls bf16 with fp32 PSUM accumulation.

Schedule: projections, attention, and the 4 chunked AllGathers are emitted
interleaved per s-chunk (attention for a q-chunk fires as soon as the K/V
chunks it reads are projected -- data-driven via `need`, so dense masks fall
back to trailing attention), and o_proj consumes gathered chunks as they
land.  All four collectives are fully hidden behind compute.  Host inputs are
pre-tiled so every DMA is contiguous per partition.

Fully-masked leading columns of diagonal blocks are clipped out of the
scores/exp/PV/ones ops (data-driven per block via first-valid-column), and
the o_proj input loads are split per s-tile so its matmuls start on the
first gathered slice.

Measured (8x trn2 NeuronCores via axon, f32 reference): HW exec ~486-508 us
per core (run-to-run variance is mostly collective rendezvous skew; TensorE
issue-bound at the chip's 13/16 HAM power state, PE idle gaps 12-27 us,
start ~21 us, drain tail ~12 us), output rel err 8.5e-3, absmax 0.042 on
out-scale 8.9.
"""

import numpy as np
import ml_dtypes

BF16 = ml_dtypes.bfloat16

B, S, H = 1, 2048, 4096
NQ, NKV, D = 32, 8, 128
NCORES = 8
QH = NQ // NCORES            # 4 q heads per core
OC = QH * D                  # 512 = per-core q/o width
KVC = (NKV // NCORES) * D    # 128 = per-core kv width
SCALING = D ** -0.5

P = 128                      # partitions
SC = 512                     # seq chunk
N_SC = S // SC               # 4
N_ST = S // P                # 16 s-tiles
N_HT = H // P                # 32 h-tiles

_CACHE = {}
TRACE = False
LAST_RESULT = None


def _classify_mask(mask):
    """mask: [S, S] additive (q, k).  Returns per-(kt, qc) class over the
    transposed [k, q] view: 'skip' | 'plain' | mask-index, plus the deduped
    multiplicative exp(mask) blocks, each [128, 512] bf16."""
    mT = np.ascontiguousarray(mask.T)  # [k, q]
    info = {}
    blocks = []
    keys = {}
    for kt in range(N_ST):
        for qc in range(N_SC):
            blk = mT[kt * P:(kt + 1) * P, qc * SC:(qc + 1) * SC]
            if np.all(blk <= -1e8):
                info[(kt, qc)] = 'skip'
            elif np.all(blk == 0.0):
                info[(kt, qc)] = 'plain'
            else:
                key = blk.tobytes()
                if key not in keys:
                    keys[key] = len(blocks)
                    blocks.append(np.exp(blk.astype(np.float64)).astype(BF16))
                # first q column with any unmasked entry: columns before it
                # are fully masked and can be clipped out of the matmuls
                valid = (blk > -1e8).any(axis=0)
                col_lo = int(np.argmax(valid)) if valid.any() else SC
                info[(kt, qc)] = (keys[key], col_lo)
    return info, blocks


def _build(binfo, n_mask):
    import concourse.bass as bass
    import concourse.bacc as bacc
    import concourse.tile as tile
    import concourse.mybir as mybir

    dt = mybir.dt
    f32, bf16 = dt.float32, dt.bfloat16
    AF = mybir.ActivationFunctionType
    ALU = mybir.AluOpType

    nc = bacc.Bacc("TRN2", target_bir_lowering=False, debug=False,
                   num_devices=NCORES)

    # pre-tiled on host for fully contiguous DMAs:
    #   x4[sc, p, o, j]  = hidden[sc*512+j, o*128+p]
    #   w6[g, p, o, j]   = wqkv_cat[o*128+p, g*128+j]   (g: q0..q3, k, v)
    #   wo3[p, o, j]     = wo_t[o*128+p, j]
    x4 = nc.dram_tensor("x4", [N_SC, P, N_HT, SC], bf16, kind="ExternalInput")
    w6 = nc.dram_tensor("w6", [6, P, N_HT, D], bf16, kind="ExternalInput")
    wo3 = nc.dram_tensor("wo3", [P, NQ * D // P, OC], bf16, kind="ExternalInput")
    cosT = nc.dram_tensor("cosT", [D, S], f32, kind="ExternalInput")
    sinT = nc.dram_tensor("sinT", [D, S], f32, kind="ExternalInput")
    rotP = nc.dram_tensor("rotP", [D, D], bf16, kind="ExternalInput")
    if n_mask:
        maskm = nc.dram_tensor("maskm", [P, n_mask * SC], bf16,
                               kind="ExternalInput")
    out = nc.dram_tensor("out", [S, OC], f32, kind="ExternalOutput")

    with tile.TileContext(nc) as tc:
        with (
            tc.tile_pool(name="consts", bufs=1) as consts,
            tc.tile_pool(name="weights", bufs=1) as weights,
            tc.tile_pool(name="qkv", bufs=1) as qkv,
            tc.tile_pool(name="xs", bufs=2) as xs,
            tc.tile_pool(name="tmp", bufs=2) as tmp,
            tc.tile_pool(name="expp", bufs=27) as expp,
            tc.tile_pool(name="att", bufs=2) as att,
            tc.tile_pool(name="outp", bufs=2) as outp,
            tc.tile_pool(name="ps_s", bufs=2, space="PSUM") as ps_s,
            tc.tile_pool(name="ps_r", bufs=1, space="PSUM") as ps_r,
            tc.tile_pool(name="ps_a", bufs=2, space="PSUM") as ps_a,
            tc.tile_pool(name="dram", bufs=1, space="DRAM") as dram,
        ):
            # DMA order matters: the first matmul needs w6[0] + the first half
            # of x chunk 0; everything else can trickle in behind it.
            w_sb = weights.tile([P, 6, N_HT, D], bf16, tag="wbig")
            nc.sync.dma_start(w_sb[:, 0, :N_HT // 2], w6.ap()[0, :, :N_HT // 2])
            nc.sync.dma_start(w_sb[:, 0, N_HT // 2:], w6.ap()[0, :, N_HT // 2:])
            x_t0 = xs.tile([P, N_HT, SC], bf16, tag="x_t", name="x_t0")
            q4 = N_HT // 4
            for qtr in range(4):
                nc.scalar.dma_start(x_t0[:, qtr * q4:(qtr + 1) * q4],
                                    x4.ap()[0, :, qtr * q4:(qtr + 1) * q4])
            for g in range(1, 6):
                nc.sync.dma_start(w_sb[:, g, :N_HT // 2], w6.ap()[g, :, :N_HT // 2])
                nc.sync.dma_start(w_sb[:, g, N_HT // 2:], w6.ap()[g, :, N_HT // 2:])

            cos_sb = consts.tile([P, S], f32)
            nc.sync.dma_start(cos_sb[:], cosT.ap())
            sin_sb = consts.tile([P, S], f32)
            nc.sync.dma_start(sin_sb[:], sinT.ap())
            rot_sb = consts.tile([P, D], bf16)
            nc.sync.dma_start(rot_sb[:], rotP.ap())
            ones_sb = consts.tile([P, P], bf16)
            nc.vector.memset(ones_sb[:], 1.0)
            if n_mask:
                mask_sb = consts.tile([P, n_mask, SC], bf16)
                nc.sync.dma_start(
                    mask_sb[:], maskm.ap().rearrange("p (n c) -> p n c", n=n_mask))

            Q_all = qkv.tile([P, QH, S], bf16)
            K_all = qkv.tile([P, S], bf16)
            V_all = qkv.tile([P, N_ST, D], bf16)

            attn_local = []
            attn_all = []
            for qc in range(N_SC):
                al = dram.tile([OC, SC], bf16, name=f"attn_local{qc}")
                ag = dram.tile([NQ * D, SC], bf16, name=f"attn_all{qc}",
                               addr_space="Shared")
                attn_local.append(al)
                attn_all.append(ag)

            # ---- interleaved: proj(sc) -> attention(ready qc) -> gather ----
            # attention for qc can only run once every K/V chunk it reads has
            # been projected; for a causal mask need[qc] == qc, for a dense
            # mask need[qc] == N_SC-1 (attention all trails the projections).
            need = {}
            for qc in range(N_SC):
                kts_q = [kt for kt in range(N_ST) if binfo[(kt, qc)] != 'skip']
                need[qc] = max(kt // (SC // P) for kt in kts_q) if kts_q else 0
            for sc in range(N_SC):
                ssl = slice(sc * SC, (sc + 1) * SC)
                if sc == 0:
                    x_t = x_t0
                else:
                    x_t = xs.tile([P, N_HT, SC], bf16, tag="x_t")
                    nc.sync.dma_start(x_t[:], x4.ap()[sc])
                # Qt (4 head-tiles) and Kt (1 tile), with RoPE
                for ot in range(QH + 1):
                    ps = ps_s.tile([P, SC], f32, tag="ps")
                    for h in range(N_HT):
                        nc.tensor.matmul(
                            ps[:], w_sb[:, ot, h, :], x_t[:, h, :],
                            start=(h == 0), stop=(h == N_HT - 1))
                    pre = tmp.tile([P, SC], bf16, tag="pre")
                    nc.vector.tensor_copy(pre[:], ps[:])
                    psr = ps_r.tile([P, SC], f32)
                    nc.tensor.matmul(psr[:], rot_sb[:], pre[:],
                                     start=True, stop=True)
                    t1 = tmp.tile([P, SC], f32, tag="t1")
                    nc.vector.tensor_tensor(t1[:], pre[:], cos_sb[:, ssl], ALU.mult)
                    t2 = tmp.tile([P, SC], f32, tag="t2")
                    nc.vector.tensor_tensor(t2[:], psr[:], sin_sb[:, ssl], ALU.mult)
                    dst = Q_all[:, ot, ssl] if ot < QH else K_all[:, ssl]
                    nc.vector.tensor_tensor(dst, t1[:], t2[:], ALU.add)
                # V in [S, D] layout
                for st in range(SC // P):
                    psv_full = ps_r.tile([P, SC], f32, tag="psr", name="psv")
                    psv = psv_full[:, :D]
                    for h in range(N_HT):
                        nc.tensor.matmul(
                            psv[:], x_t[:, h, st * P:(st + 1) * P],
                            w_sb[:, 5, h, :],
                            start=(h == 0), stop=(h == N_HT - 1))
                    nc.vector.tensor_copy(V_all[:, sc * (SC // P) + st, :], psv[:])

                # ---- attention for every qc whose K/V chunks are now ready --
                # one-head software pipeline: emit head h+1's scores before
                # head h's PV so the PE has work while ScalarE runs the exps.
                ready = [q for q in range(N_SC) if need[q] == sc]

                def emit_pv(head, kts, exp_tiles, qc):
                    psa = ps_a.tile([P, SC], f32, tag="pv", name="psa",
                                    bufs=3)
                    pso = ps_a.tile([P, SC], f32, tag="ones", name="pso")
                    for i, kt in enumerate(kts):
                        ex, lo = exp_tiles[kt]
                        n = SC - lo
                        nc.tensor.matmul(psa[:, lo:], V_all[:, kt, :], ex[:, :n],
                                         start=(i == 0), stop=(i == len(kts) - 1))
                        nc.tensor.matmul(pso[:, lo:], ones_sb[:], ex[:, :n],
                                         start=(i == 0), stop=(i == len(kts) - 1))
                    # copy PV out of PSUM quickly so the bank frees; divide later
                    acop = att.tile([P, SC], f32, tag="acop", name="acop")
                    nc.vector.tensor_copy(acop[:], psa[:])
                    rec = tmp.tile([P, SC], f32, tag="rec", name="rec")
                    nc.vector.reciprocal(rec[:], pso[:])
                    a_sb = att.tile([P, SC], bf16, tag="a_sb", name="a_sb")
                    nc.vector.tensor_tensor(a_sb[:], acop[:], rec[:], ALU.mult)
                    nc.sync.dma_start(
                        attn_local[qc][head * P:(head + 1) * P, :], a_sb[:])

                for qc in ready:
                    qsl = slice(qc * SC, (qc + 1) * SC)
                    pend = None
                    for head in range(QH):
                        kts = [kt for kt in range(N_ST)
                               if binfo[(kt, qc)] != 'skip']
                        exp_tiles = {}
                        for kt in kts:
                            cls = binfo[(kt, qc)]
                            lo = 0 if cls == 'plain' else cls[1]
                            n = SC - lo
                            pss = ps_s.tile([P, SC], f32, tag="ps")
                            nc.tensor.matmul(
                                pss[:, :n], K_all[:, kt * P:(kt + 1) * P],
                                Q_all[:, head, qc * SC + lo:(qc + 1) * SC],
                                start=True, stop=True)
                            ex = expp.tile([P, SC], bf16, tag="ex")
                            nc.scalar.activation(ex[:, :n], pss[:, :n], AF.Exp,
                                                 scale=float(SCALING))
                            if cls != 'plain':
                                nc.vector.tensor_tensor(
                                    ex[:, :n], ex[:, :n],
                                    mask_sb[:, cls[0], lo:], ALU.mult)
                            exp_tiles[kt] = (ex, lo)
                        if pend is not None:
                            emit_pv(*pend)
                        pend = (head, kts, exp_tiles, qc)
                    emit_pv(*pend)
                    nc.gpsimd.collective_compute(
                        "AllGather", mybir.AluOpType.bypass,
                        replica_groups=[list(range(NCORES))],
                        ins=[attn_local[qc].opt()], outs=[attn_all[qc].opt()])

            # ---------------- o_proj slice, per gathered chunk ----------------
            wo_sb = weights.tile([P, NQ * D // P, OC], bf16, tag="wbig")
            nc.sync.dma_start(wo_sb[:], wo3.ap())
            n_at = NQ * D // P          # 32 feature tiles
            for qc in range(N_SC):
                a_t = xs.tile([P, n_at, SC], bf16, tag="x_t")
                src_r = attn_all[qc].rearrange("(o p) s -> p o s", p=P)
                for st2 in range(SC // P):
                    nc.sync.dma_start(
                        a_t[:, :, st2 * P:(st2 + 1) * P],
                        src_r[:, :, st2 * P:(st2 + 1) * P])
                for st2 in range(SC // P):
                    pso2 = ps_s.tile([P, SC], f32, tag="ps", name="pso2")
                    for kt in range(n_at):
                        nc.tensor.matmul(
                            pso2[:], a_t[:, kt, st2 * P:(st2 + 1) * P],
                            wo_sb[:, kt, :],
                            start=(kt == 0), stop=(kt == n_at - 1))
                    o_sb = outp.tile([P, OC], f32)
                    nc.vector.tensor_copy(o_sb[:], pso2[:])
                    nc.sync.dma_start(
                        out.ap()[(qc * (SC // P) + st2) * P:
                                 (qc * (SC // P) + st2 + 1) * P, :], o_sb[:])

    nc.compile()
    return nc


def _prep_inputs(hidden_states, cos, sin, attention_mask, wq, wk, wv, wo,
                 mask_blocks):
    hs = hidden_states[0]  # [S, H] f32
    # x4[sc, p, o, j] = hs[sc*512+j, o*128+p]
    x4 = np.ascontiguousarray(
        hs.reshape(N_SC, SC, N_HT, P).transpose(0, 3, 2, 1)).astype(BF16)
    cosT = np.ascontiguousarray(cos[0].T).astype(np.float32)
    sinT = np.ascontiguousarray(sin[0].T).astype(np.float32)
    # rot(q)[m] = -q[m+64] for m<64 ; +q[m-64] for m>=64.
    # matmul computes out[m] = sum_k lhsT[k, m] q[k]
    rot = np.zeros((D, D), np.float32)
    half = D // 2
    for m in range(half):
        rot[m + half, m] = -1.0
    for m in range(half, D):
        rot[m - half, m] = 1.0
    rotP = rot.astype(BF16)
    if mask_blocks:
        maskm = np.concatenate(mask_blocks, axis=1).astype(BF16)
        maskm = np.ascontiguousarray(maskm)
    in_maps = []
    for c in range(NCORES):
        wq_s = wq[c * OC:(c + 1) * OC, :]
        wk_s = wk[c * KVC:(c + 1) * KVC, :]
        wv_s = wv[c * KVC:(c + 1) * KVC, :]
        # wqkv_cat.T: [H, 768]; w6[g, p, o, j] = wqkv_cat_T[o*128+p, g*128+j]
        wcat = np.concatenate([wq_s, wk_s, wv_s], axis=0).T  # [H, 768]
        w6 = np.ascontiguousarray(
            wcat.reshape(N_HT, P, 6, D).transpose(2, 1, 0, 3)).astype(BF16)
        # wo3[p, o, j] = wo_t[o*128+p, j],  wo_t = wo[cOC:(c+1)OC, :].T [4096, 512]
        wo_c = wo[c * OC:(c + 1) * OC, :].T  # [4096, 512]
        wo3 = np.ascontiguousarray(
            wo_c.reshape(NQ * D // P, P, OC).transpose(1, 0, 2)).astype(BF16)
        m = {"x4": x4, "w6": w6, "wo3": wo3, "cosT": cosT,
             "sinT": sinT, "rotP": rotP}
        if mask_blocks:
            m["maskm"] = maskm
        in_maps.append(m)
    return in_maps


def kernel(hidden_states, cos, sin, attention_mask, wq, wk, wv, wo):
    global LAST_RESULT
    from concourse.bass_utils import run_bass_kernel_spmd

    hidden_states = np.asarray(hidden_states, dtype=np.float32)
    cos = np.asarray(cos, dtype=np.float32)
    sin = np.asarray(sin, dtype=np.float32)
    attention_mask = np.asarray(attention_mask, dtype=np.float32)
    wq = np.asarray(wq, dtype=np.float32)
    wk = np.asarray(wk, dtype=np.float32)
    wv = np.asarray(wv, dtype=np.float32)
    wo = np.asarray(wo, dtype=np.float32)

    binfo, mask_blocks = _classify_mask(attention_mask[0, 0])
    key = tuple(sorted(binfo.items())) + (len(mask_blocks),)
    if key not in _CACHE:
        _CACHE[key] = _build(binfo, len(mask_blocks))
    nc = _CACHE[key]

    in_maps = _prep_inputs(hidden_states, cos, sin, attention_mask,
                           wq, wk, wv, wo, mask_blocks)
    res = run_bass_kernel_spmd(nc, in_maps, core_ids=list(range(NCORES)),
                               trace=TRACE)
    LAST_RESULT = res
    out_full = np.empty((S, NQ * D), np.float32)
    for c in range(NCORES):
        out_full[:, c * OC:(c + 1) * OC] = res.results[c]["out"]
    return out_full.reshape(B, S, NQ * D)


# revision 43
# speedup vs baseline: 1.0046x; 1.0046x over previous
"""Trainium2 Bass kernel for nn_MistralAttentionQ (B=1, S=2048, H=4096, 32 Q heads,
8 KV heads, D=128), tensor-parallel over heads across 8 NeuronCores.

Per core c: 4 Q heads (rows 512c:512c+512 of wq) + 1 KV head (rows 128c:128c+128
of wk/wv).  Everything is kept in transposed [feature, seq] layout so that no
on-device activation transposes are needed:

  - Qt/Kt projections:  out[d_tile, s_chunk] = wqkv_t_tile.T @ xT_chunk
  - RoPE rotate_half:   rot(Q) = Prot @ Qt  (tiny 128x128 matmul), then
                        q' = q*cosT + rot(q)*sinT on VectorE
  - V projection:       V[s_tile, d] = xT_tile.T @ wv_t  ([S, D] layout)
  - scores (transposed): sT[k_tile, q_chunk] = Kt_tile.T @ Qt_chunk
  - softmax: exp on ScalarE (scale folded in, no max subtraction -- scores are
    bounded ~ +-10 for this problem); row sums via an all-ones matmul on the
    TensorEngine (partition-dim reduce); masking is multiplicative exp(mask)
    applied only to "mixed" blocks; fully-masked blocks are skipped.
  - PV:                 attn_t[d, q_chunk] += V_ktile.T @ expT[k_tile, q_chunk]
  - AllGather attn_t shards -> [4096, 2048], then o_proj slice:
                        out[s_tile, 512c:512c+512] = attn_all_tile.T @ wo_t
All matmuls bf16 with fp32 PSUM accumulation.

Schedule: projections, attention, and the 4 chunked AllGathers are emitted
interleaved per s-chunk (attention for a q-chunk fires as soon as the K/V
chunks it reads are projected -- data-driven via `need`, so dense masks fall
back to trailing attention), and o_proj consumes gathered chunks as they
land.  All four collectives are fully hidden behind compute.  Host inputs are
pre-tiled so every DMA is contiguous per partition.

Fully-masked leading columns of diagonal blocks are clipped out of the
scores/exp/PV/ones ops (data-driven per block via first-valid-column), and
the o_proj input loads are split per s-tile so its matmuls start on the
first gathered slice.

Measured (8x trn2 NeuronCores via axon, f32 reference): HW exec ~486-508 us
per core (run-to-run variance is mostly collective rendezvous skew; TensorE
issue-bound at the chip's 13/16 HAM power state, PE idle gaps 12-27 us,
start ~21 us, drain tail ~12 us), output rel err 8.5e-3, absmax 0.042 on
out-scale 8.9.
"""

import numpy as np
import ml_dtypes

BF16 = ml_dtypes.bfloat16

B, S, H = 1, 2048, 4096
NQ, NKV, D = 32, 8, 128
NCORES = 8
QH = NQ // NCORES            # 4 q heads per core
OC = QH * D                  # 512 = per-core q/o width
KVC = (NKV // NCORES) * D    # 128 = per-core kv width
SCALING = D ** -0.5

P = 128                      # partitions
SC = 512                     # seq chunk
N_SC = S // SC               # 4
N_ST = S // P                # 16 s-tiles
N_HT = H // P                # 32 h-tiles

_CACHE = {}
TRACE = False
LAST_RESULT = None


def _classify_mask(mask):
    """mask: [S, S] additive (q, k).  Returns per-(kt, qc) class over the
    transposed [k, q] view: 'skip' | 'plain' | mask-index, plus the deduped
    multiplicative exp(mask) blocks, each [128, 512] bf16."""
    mT = np.ascontiguousarray(mask.T)  # [k, q]
    info = {}
    blocks = []
    keys = {}
    for kt in range(N_ST):
        for qc in range(N_SC):
            blk = mT[kt * P:(kt + 1) * P, qc * SC:(qc + 1) * SC]
            if np.all(blk <= -1e8):
                info[(kt, qc)] = 'skip'
            elif np.all(blk == 0.0):
                info[(kt, qc)] = 'plain'
            else:
                key = blk.tobytes()
                if key not in keys:
                    keys[key] = len(blocks)
                    blocks.append(np.exp(blk.astype(np.float64)).astype(BF16))
                # first q column with any unmasked entry: columns before it
                # are fully masked and can be clipped out of the matmuls
                valid = (blk > -1e8).any(axis=0)
                col_lo = int(np.argmax(valid)) if valid.any() else SC
                info[(kt, qc)] = (keys[key], col_lo)
    return info, blocks


def _build(binfo, n_mask):
    import concourse.bass as bass
    import concourse.bacc as bacc
    import concourse.tile as tile
    import concourse.mybir as mybir

    dt = mybir.dt
    f32, bf16 = dt.float32, dt.bfloat16
    AF = mybir.ActivationFunctionType
    ALU = mybir.AluOpType

    nc = bacc.Bacc("TRN2", target_bir_lowering=False, debug=False,
                   num_devices=NCORES)

    # pre-tiled on host for fully contiguous DMAs:
    #   x4[sc, p, o, j]  = hidden[sc*512+j, o*128+p]
    #   w6[g, p, o, j]   = wqkv_cat[o*128+p, g*128+j]   (g: q0..q3, k, v)
    #   wo3[p, o, j]     = wo_t[o*128+p, j]
    x4 = nc.dram_tensor("x4", [N_SC, P, N_HT, SC], bf16, kind="ExternalInput")
    w6 = nc.dram_tensor("w6", [6, P, N_HT, D], bf16, kind="ExternalInput")
    wo3 = nc.dram_tensor("wo3", [P, NQ * D // P, OC], bf16, kind="ExternalInput")
    cosT = nc.dram_tensor("cosT", [D, S], f32, kind="ExternalInput")
    sinT = nc.dram_tensor("sinT", [D, S], f32, kind="ExternalInput")
    rotP = nc.dram_tensor("rotP", [D, D], bf16, kind="ExternalInput")
    if n_mask:
        maskm = nc.dram_tensor("maskm", [P, n_mask * SC], bf16,
                               kind="ExternalInput")
    out = nc.dram_tensor("out", [S, OC], f32, kind="ExternalOutput")

    with tile.TileContext(nc) as tc:
        with (
            tc.tile_pool(name="consts", bufs=1) as consts,
            tc.tile_pool(name="weights", bufs=1) as weights,
            tc.tile_pool(name="qkv", bufs=1) as qkv,
            tc.tile_pool(name="xs", bufs=2) as xs,
            tc.tile_pool(name="tmp", bufs=2) as tmp,
            tc.tile_pool(name="expp", bufs=27) as expp,
            tc.tile_pool(name="att", bufs=2) as att,
            tc.tile_pool(name="outp", bufs=2) as outp,
            tc.tile_pool(name="ps_s", bufs=3, space="PSUM") as ps_s,
            tc.tile_pool(name="ps_r", bufs=1, space="PSUM") as ps_r,
            tc.tile_pool(name="ps_a", bufs=2, space="PSUM") as ps_a,
            tc.tile_pool(name="dram", bufs=1, space="DRAM") as dram,
        ):
            # DMA order matters: the first matmul needs w6[0] + the first half
            # of x chunk 0; everything else can trickle in behind it.
            w_sb = weights.tile([P, 6, N_HT, D], bf16, tag="wbig")
            nc.sync.dma_start(w_sb[:, 0, :N_HT // 2], w6.ap()[0, :, :N_HT // 2])
            nc.sync.dma_start(w_sb[:, 0, N_HT // 2:], w6.ap()[0, :, N_HT // 2:])
            x_t0 = xs.tile([P, N_HT, SC], bf16, tag="x_t", name="x_t0")
            q4 = N_HT // 4
            for qtr in range(4):
                nc.scalar.dma_start(x_t0[:, qtr * q4:(qtr + 1) * q4],
                                    x4.ap()[0, :, qtr * q4:(qtr + 1) * q4])
            # rope(sc0) needs cos/sin chunk 0 by ~23us -- interleave those
            # quarters ahead of the remaining weight groups on the sync queue
            cos_sb = consts.tile([P, S], f32)
            sin_sb = consts.tile([P, S], f32)
            nc.sync.dma_start(cos_sb[:, :SC], cosT.ap()[:, :SC])
            nc.sync.dma_start(sin_sb[:, :SC], sinT.ap()[:, :SC])
            for g in range(1, 6):
                nc.sync.dma_start(w_sb[:, g, :N_HT // 2], w6.ap()[g, :, :N_HT // 2])
                nc.sync.dma_start(w_sb[:, g, N_HT // 2:], w6.ap()[g, :, N_HT // 2:])
            for scq in range(1, N_SC):
                nc.sync.dma_start(cos_sb[:, scq * SC:(scq + 1) * SC],
                                  cosT.ap()[:, scq * SC:(scq + 1) * SC])
                nc.sync.dma_start(sin_sb[:, scq * SC:(scq + 1) * SC],
                                  sinT.ap()[:, scq * SC:(scq + 1) * SC])
            rot_sb = consts.tile([P, D], bf16)
            nc.sync.dma_start(rot_sb[:], rotP.ap())
            ones_sb = consts.tile([P, P], bf16)
            nc.vector.memset(ones_sb[:], 1.0)
            if n_mask:
                mask_sb = consts.tile([P, n_mask, SC], bf16)
                nc.sync.dma_start(
                    mask_sb[:], maskm.ap().rearrange("p (n c) -> p n c", n=n_mask))

            Q_all = qkv.tile([P, QH, S], bf16)
            K_all = qkv.tile([P, S], bf16)
            V_all = qkv.tile([P, N_ST, D], bf16)

            attn_local = []
            attn_all = []
            for qc in range(N_SC):
                al = dram.tile([OC, SC], bf16, name=f"attn_local{qc}")
                ag = dram.tile([NQ * D, SC], bf16, name=f"attn_all{qc}",
                               addr_space="Shared")
                attn_local.append(al)
                attn_all.append(ag)

            # ---- interleaved: proj(sc) -> attention(ready qc) -> gather ----
            # attention for qc can only run once every K/V chunk it reads has
            # been projected; for a causal mask need[qc] == qc, for a dense
            # mask need[qc] == N_SC-1 (attention all trails the projections).
            need = {}
            for qc in range(N_SC):
                kts_q = [kt for kt in range(N_ST) if binfo[(kt, qc)] != 'skip']
                need[qc] = max(kt // (SC // P) for kt in kts_q) if kts_q else 0
            for sc in range(N_SC):
                ssl = slice(sc * SC, (sc + 1) * SC)
                if sc == 0:
                    x_t = x_t0
                else:
                    x_t = xs.tile([P, N_HT, SC], bf16, tag="x_t")
                    nc.sync.dma_start(x_t[:], x4.ap()[sc])
                # Qt (4 head-tiles) and Kt (1 tile), with RoPE
                for ot in range(QH + 1):
                    ps = ps_s.tile([P, SC], f32, tag="ps")
                    for h in range(N_HT):
                        nc.tensor.matmul(
                            ps[:], w_sb[:, ot, h, :], x_t[:, h, :],
                            start=(h == 0), stop=(h == N_HT - 1))
                    pre = tmp.tile([P, SC], bf16, tag="pre")
                    nc.vector.tensor_copy(pre[:], ps[:])
                    psr = ps_r.tile([P, SC], f32)
                    nc.tensor.matmul(psr[:], rot_sb[:], pre[:],
                                     start=True, stop=True)
                    t1 = tmp.tile([P, SC], f32, tag="t1")
                    nc.vector.tensor_tensor(t1[:], pre[:], cos_sb[:, ssl], ALU.mult)
                    t2 = tmp.tile([P, SC], f32, tag="t2")
                    nc.vector.tensor_tensor(t2[:], psr[:], sin_sb[:, ssl], ALU.mult)
                    dst = Q_all[:, ot, ssl] if ot < QH else K_all[:, ssl]
                    nc.vector.tensor_tensor(dst, t1[:], t2[:], ALU.add)
                # V in [S, D] layout
                for st in range(SC // P):
                    psv_full = ps_r.tile([P, SC], f32, tag="psr", name="psv")
                    psv = psv_full[:, :D]
                    for h in range(N_HT):
                        nc.tensor.matmul(
                            psv[:], x_t[:, h, st * P:(st + 1) * P],
                            w_sb[:, 5, h, :],
                            start=(h == 0), stop=(h == N_HT - 1))
                    nc.vector.tensor_copy(V_all[:, sc * (SC // P) + st, :], psv[:])

                # ---- attention for every qc whose K/V chunks are now ready --
                # one-head software pipeline: emit head h+1's scores before
                # head h's PV so the PE has work while ScalarE runs the exps.
                ready = [q for q in range(N_SC) if need[q] == sc]

                def emit_pv(head, kts, exp_tiles, qc):
                    psa = ps_a.tile([P, SC], f32, tag="pv", name="psa")
                    pso = ps_a.tile([P, SC], f32, tag="ones", name="pso")
                    for i, kt in enumerate(kts):
                        ex, lo = exp_tiles[kt]
                        n = SC - lo
                        nc.tensor.matmul(psa[:, lo:], V_all[:, kt, :], ex[:, :n],
                                         start=(i == 0), stop=(i == len(kts) - 1))
                        nc.tensor.matmul(pso[:, lo:], ones_sb[:], ex[:, :n],
                                         start=(i == 0), stop=(i == len(kts) - 1))
                    # copy PV out of PSUM quickly so the bank frees; divide later
                    acop = att.tile([P, SC], f32, tag="acop", name="acop")
                    nc.vector.tensor_copy(acop[:], psa[:])
                    rec = tmp.tile([P, SC], f32, tag="rec", name="rec")
                    nc.vector.reciprocal(rec[:], pso[:])
                    a_sb = att.tile([P, SC], bf16, tag="a_sb", name="a_sb")
                    nc.vector.tensor_tensor(a_sb[:], acop[:], rec[:], ALU.mult)
                    nc.sync.dma_start(
                        attn_local[qc][head * P:(head + 1) * P, :], a_sb[:])

                for qc in ready:
                    qsl = slice(qc * SC, (qc + 1) * SC)
                    pend = None
                    for head in range(QH):
                        kts = [kt for kt in range(N_ST)
                               if binfo[(kt, qc)] != 'skip']
                        exp_tiles = {}
                        for kt in kts:
                            cls = binfo[(kt, qc)]
                            lo = 0 if cls == 'plain' else cls[1]
                            n = SC - lo
                            pss = ps_s.tile([P, SC], f32, tag="ps")
                            nc.tensor.matmul(
                                pss[:, :n], K_all[:, kt * P:(kt + 1) * P],
                                Q_all[:, head, qc * SC + lo:(qc + 1) * SC],
                                start=True, stop=True)
                            ex = expp.tile([P, SC], bf16, tag="ex")
                            nc.scalar.activation(ex[:, :n], pss[:, :n], AF.Exp,
                                                 scale=float(SCALING))
                            if cls != 'plain':
                                nc.vector.tensor_tensor(
                                    ex[:, :n], ex[:, :n],
                                    mask_sb[:, cls[0], lo:], ALU.mult)
                            exp_tiles[kt] = (ex, lo)
                        if pend is not None:
                            emit_pv(*pend)
                        pend = (head, kts, exp_tiles, qc)
                    emit_pv(*pend)
                    nc.gpsimd.collective_compute(
                        "AllGather", mybir.AluOpType.bypass,
                        replica_groups=[list(range(NCORES))],
                        ins=[attn_local[qc].opt()], outs=[attn_all[qc].opt()])

            # ---------------- o_proj slice, per gathered chunk ----------------
            wo_sb = weights.tile([P, NQ * D // P, OC], bf16, tag="wbig")
            nc.sync.dma_start(wo_sb[:], wo3.ap())
            n_at = NQ * D // P          # 32 feature tiles
            for qc in range(N_SC):
                a_t = xs.tile([P, n_at, SC], bf16, tag="x_t")
                src_r = attn_all[qc].rearrange("(o p) s -> p o s", p=P)
                for st2 in range(SC // P):
                    nc.sync.dma_start(
                        a_t[:, :, st2 * P:(st2 + 1) * P],
                        src_r[:, :, st2 * P:(st2 + 1) * P])
                for st2 in range(SC // P):
                    pso2 = ps_s.tile([P, SC], f32, tag="ps", name="pso2")
                    for kt in range(n_at):
                        nc.tensor.matmul(
                            pso2[:], a_t[:, kt, st2 * P:(st2 + 1) * P],
                            wo_sb[:, kt, :],
                            start=(kt == 0), stop=(kt == n_at - 1))
                    o_sb = outp.tile([P, OC], f32)
                    nc.vector.tensor_copy(o_sb[:], pso2[:])
                    nc.sync.dma_start(
                        out.ap()[(qc * (SC // P) + st2) * P:
                                 (qc * (SC // P) + st2 + 1) * P, :], o_sb[:])

    nc.compile()
    return nc


def _prep_inputs(hidden_states, cos, sin, attention_mask, wq, wk, wv, wo,
                 mask_blocks):
    hs = hidden_states[0]  # [S, H] f32
    # x4[sc, p, o, j] = hs[sc*512+j, o*128+p]
    x4 = np.ascontiguousarray(
        hs.reshape(N_SC, SC, N_HT, P).transpose(0, 3, 2, 1)).astype(BF16)
    cosT = np.ascontiguousarray(cos[0].T).astype(np.float32)
    sinT = np.ascontiguousarray(sin[0].T).astype(np.float32)
    # rot(q)[m] = -q[m+64] for m<64 ; +q[m-64] for m>=64.
    # matmul computes out[m] = sum_k lhsT[k, m] q[k]
    rot = np.zeros((D, D), np.float32)
    half = D // 2
    for m in range(half):
        rot[m + half, m] = -1.0
    for m in range(half, D):
        rot[m - half, m] = 1.0
    rotP = rot.astype(BF16)
    if mask_blocks:
        maskm = np.concatenate(mask_blocks, axis=1).astype(BF16)
        maskm = np.ascontiguousarray(maskm)
    in_maps = []
    for c in range(NCORES):
        wq_s = wq[c * OC:(c + 1) * OC, :]
        wk_s = wk[c * KVC:(c + 1) * KVC, :]
        wv_s = wv[c * KVC:(c + 1) * KVC, :]
        # wqkv_cat.T: [H, 768]; w6[g, p, o, j] = wqkv_cat_T[o*128+p, g*128+j]
        wcat = np.concatenate([wq_s, wk_s, wv_s], axis=0).T  # [H, 768]
        w6 = np.ascontiguousarray(
            wcat.reshape(N_HT, P, 6, D).transpose(2, 1, 0, 3)).astype(BF16)
        # wo3[p, o, j] = wo_t[o*128+p, j],  wo_t = wo[cOC:(c+1)OC, :].T [4096, 512]
        wo_c = wo[c * OC:(c + 1) * OC, :].T  # [4096, 512]
        wo3 = np.ascontiguousarray(
            wo_c.reshape(NQ * D // P, P, OC).transpose(1, 0, 2)).astype(BF16)
        m = {"x4": x4, "w6": w6, "wo3": wo3, "cosT": cosT,
             "sinT": sinT, "rotP": rotP}
        if mask_blocks:
            m["maskm"] = maskm
        in_maps.append(m)
    return in_maps


def kernel(hidden_states, cos, sin, attention_mask, wq, wk, wv, wo):
    global LAST_RESULT
    from concourse.bass_utils import run_bass_kernel_spmd

    hidden_states = np.asarray(hidden_states, dtype=np.float32)
    cos = np.asarray(cos, dtype=np.float32)
    sin = np.asarray(sin, dtype=np.float32)
    attention_mask = np.asarray(attention_mask, dtype=np.float32)
    wq = np.asarray(wq, dtype=np.float32)
    wk = np.asarray(wk, dtype=np.float32)
    wv = np.asarray(wv, dtype=np.float32)
    wo = np.asarray(wo, dtype=np.float32)

    binfo, mask_blocks = _classify_mask(attention_mask[0, 0])
    key = tuple(sorted(binfo.items())) + (len(mask_blocks),)
    if key not in _CACHE:
        _CACHE[key] = _build(binfo, len(mask_blocks))
    nc = _CACHE[key]

    in_maps = _prep_inputs(hidden_states, cos, sin, attention_mask,
                           wq, wk, wv, wo, mask_blocks)
    res = run_bass_kernel_spmd(nc, in_maps, core_ids=list(range(NCORES)),
                               trace=TRACE)
    LAST_RESULT = res
    out_full = np.empty((S, NQ * D), np.float32)
    for c in range(NCORES):
        out_full[:, c * OC:(c + 1) * OC] = res.results[c]["out"]
    return out_full.reshape(B, S, NQ * D)


# revision 44
# speedup vs baseline: 1.0110x; 1.0063x over previous
"""Trainium2 Bass kernel for nn_MistralAttentionQ (B=1, S=2048, H=4096, 32 Q heads,
8 KV heads, D=128), tensor-parallel over heads across 8 NeuronCores.

Per core c: 4 Q heads (rows 512c:512c+512 of wq) + 1 KV head (rows 128c:128c+128
of wk/wv).  Everything is kept in transposed [feature, seq] layout so that no
on-device activation transposes are needed:

  - Qt/Kt projections:  out[d_tile, s_chunk] = wqkv_t_tile.T @ xT_chunk
  - RoPE rotate_half:   rot(Q) = Prot @ Qt  (tiny 128x128 matmul), then
                        q' = q*cosT + rot(q)*sinT on VectorE
  - V projection:       V[s_tile, d] = xT_tile.T @ wv_t  ([S, D] layout)
  - scores (transposed): sT[k_tile, q_chunk] = Kt_tile.T @ Qt_chunk
  - softmax: exp on ScalarE (scale folded in, no max subtraction -- scores are
    bounded ~ +-10 for this problem); row sums via an all-ones matmul on the
    TensorEngine (partition-dim reduce); masking is multiplicative exp(mask)
    applied only to "mixed" blocks; fully-masked blocks are skipped.
  - PV:                 attn_t[d, q_chunk] += V_ktile.T @ expT[k_tile, q_chunk]
  - AllGather attn_t shards -> [4096, 2048], then o_proj slice:
                        out[s_tile, 512c:512c+512] = attn_all_tile.T @ wo_t
All matmuls bf16 with fp32 PSUM accumulation.

Schedule: projections, attention, and the 4 chunked AllGathers are emitted
interleaved per s-chunk (attention for a q-chunk fires as soon as the K/V
chunks it reads are projected -- data-driven via `need`, so dense masks fall
back to trailing attention), and o_proj consumes gathered chunks as they
land.  All four collectives are fully hidden behind compute.  Host inputs are
pre-tiled so every DMA is contiguous per partition.

Fully-masked leading columns of diagonal blocks are clipped out of the
scores/exp/PV/ones ops (data-driven per block via first-valid-column), and
the o_proj input loads are split per s-tile so its matmuls start on the
first gathered slice.

Measured (8x trn2 NeuronCores via axon, f32 reference): HW exec ~486-508 us
per core (run-to-run variance is mostly collective rendezvous skew; TensorE
issue-bound at the chip's 13/16 HAM power state, PE idle gaps 12-27 us,
start ~21 us, drain tail ~12 us), output rel err 8.5e-3, absmax 0.042 on
out-scale 8.9.
"""

import numpy as np
import ml_dtypes

BF16 = ml_dtypes.bfloat16

B, S, H = 1, 2048, 4096
NQ, NKV, D = 32, 8, 128
NCORES = 8
QH = NQ // NCORES            # 4 q heads per core
OC = QH * D                  # 512 = per-core q/o width
KVC = (NKV // NCORES) * D    # 128 = per-core kv width
SCALING = D ** -0.5

P = 128                      # partitions
SC = 512                     # seq chunk
N_SC = S // SC               # 4
N_ST = S // P                # 16 s-tiles
N_HT = H // P                # 32 h-tiles

_CACHE = {}
TRACE = False
LAST_RESULT = None


def _classify_mask(mask):
    """mask: [S, S] additive (q, k).  Returns per-(kt, qc) class over the
    transposed [k, q] view: 'skip' | 'plain' | mask-index, plus the deduped
    multiplicative exp(mask) blocks, each [128, 512] bf16."""
    mT = np.ascontiguousarray(mask.T)  # [k, q]
    info = {}
    blocks = []
    keys = {}
    for kt in range(N_ST):
        for qc in range(N_SC):
            blk = mT[kt * P:(kt + 1) * P, qc * SC:(qc + 1) * SC]
            if np.all(blk <= -1e8):
                info[(kt, qc)] = 'skip'
            elif np.all(blk == 0.0):
                info[(kt, qc)] = 'plain'
            else:
                key = blk.tobytes()
                if key not in keys:
                    keys[key] = len(blocks)
                    blocks.append(np.exp(blk.astype(np.float64)).astype(BF16))
                # first q column with any unmasked entry: columns before it
                # are fully masked and can be clipped out of the matmuls
                valid = (blk > -1e8).any(axis=0)
                col_lo = int(np.argmax(valid)) if valid.any() else SC
                info[(kt, qc)] = (keys[key], col_lo)
    return info, blocks


def _build(binfo, n_mask):
    import concourse.bass as bass
    import concourse.bacc as bacc
    import concourse.tile as tile
    import concourse.mybir as mybir

    dt = mybir.dt
    f32, bf16 = dt.float32, dt.bfloat16
    AF = mybir.ActivationFunctionType
    ALU = mybir.AluOpType

    nc = bacc.Bacc("TRN2", target_bir_lowering=False, debug=False,
                   num_devices=NCORES)

    # pre-tiled on host for fully contiguous DMAs:
    #   x4[sc, p, o, j]  = hidden[sc*512+j, o*128+p]
    #   w6[g, p, o, j]   = wqkv_cat[o*128+p, g*128+j]   (g: q0..q3, k, v)
    #   wo3[p, o, j]     = wo_t[o*128+p, j]
    x4 = nc.dram_tensor("x4", [N_SC, P, N_HT, SC], bf16, kind="ExternalInput")
    w6 = nc.dram_tensor("w6", [6, P, N_HT, D], bf16, kind="ExternalInput")
    wo3 = nc.dram_tensor("wo3", [P, NQ * D // P, OC], bf16, kind="ExternalInput")
    cosT = nc.dram_tensor("cosT", [D, S], f32, kind="ExternalInput")
    sinT = nc.dram_tensor("sinT", [D, S], f32, kind="ExternalInput")
    rotP = nc.dram_tensor("rotP", [D, D], bf16, kind="ExternalInput")
    if n_mask:
        maskm = nc.dram_tensor("maskm", [P, n_mask * SC], bf16,
                               kind="ExternalInput")
    out = nc.dram_tensor("out", [S, OC], f32, kind="ExternalOutput")

    with tile.TileContext(nc) as tc:
        with (
            tc.tile_pool(name="consts", bufs=1) as consts,
            tc.tile_pool(name="weights", bufs=1) as weights,
            tc.tile_pool(name="qkv", bufs=1) as qkv,
            tc.tile_pool(name="xs", bufs=2) as xs,
            tc.tile_pool(name="tmp", bufs=2) as tmp,
            tc.tile_pool(name="expp", bufs=27) as expp,
            tc.tile_pool(name="att", bufs=2) as att,
            tc.tile_pool(name="outp", bufs=2) as outp,
            tc.tile_pool(name="ps_s", bufs=3, space="PSUM") as ps_s,
            tc.tile_pool(name="ps_r", bufs=1, space="PSUM") as ps_r,
            tc.tile_pool(name="ps_a", bufs=2, space="PSUM") as ps_a,
            tc.tile_pool(name="dram", bufs=1, space="DRAM") as dram,
        ):
            # DMA order matters: the first matmul needs w6[0] + the first half
            # of x chunk 0; everything else can trickle in behind it.
            w_sb = weights.tile([P, 6, N_HT, D], bf16, tag="wbig")
            nc.sync.dma_start(w_sb[:, 0, :N_HT // 2], w6.ap()[0, :, :N_HT // 2])
            nc.sync.dma_start(w_sb[:, 0, N_HT // 2:], w6.ap()[0, :, N_HT // 2:])
            x_t0 = xs.tile([P, N_HT, SC], bf16, tag="x_t", name="x_t0")
            q4 = N_HT // 4
            for qtr in range(4):
                nc.scalar.dma_start(x_t0[:, qtr * q4:(qtr + 1) * q4],
                                    x4.ap()[0, :, qtr * q4:(qtr + 1) * q4])
            # rope(sc0) needs cos/sin chunk 0 by ~23us -- interleave those
            # quarters ahead of the remaining weight groups on the sync queue
            cos_sb = consts.tile([P, S], f32)
            sin_sb = consts.tile([P, S], f32)
            nc.sync.dma_start(cos_sb[:, :SC], cosT.ap()[:, :SC])
            nc.sync.dma_start(sin_sb[:, :SC], sinT.ap()[:, :SC])
            for g in range(1, 6):
                nc.sync.dma_start(w_sb[:, g, :N_HT // 2], w6.ap()[g, :, :N_HT // 2])
                nc.sync.dma_start(w_sb[:, g, N_HT // 2:], w6.ap()[g, :, N_HT // 2:])
            for scq in range(1, N_SC):
                nc.sync.dma_start(cos_sb[:, scq * SC:(scq + 1) * SC],
                                  cosT.ap()[:, scq * SC:(scq + 1) * SC])
                nc.sync.dma_start(sin_sb[:, scq * SC:(scq + 1) * SC],
                                  sinT.ap()[:, scq * SC:(scq + 1) * SC])
            rot_sb = consts.tile([P, D], bf16)
            nc.sync.dma_start(rot_sb[:], rotP.ap())
            ones_sb = consts.tile([P, P], bf16)
            nc.vector.memset(ones_sb[:], 1.0)
            if n_mask:
                mask_sb = consts.tile([P, n_mask, SC], bf16)
                nc.sync.dma_start(
                    mask_sb[:], maskm.ap().rearrange("p (n c) -> p n c", n=n_mask))

            Q_all = qkv.tile([P, QH, S], bf16)
            K_all = qkv.tile([P, S], bf16)
            V_all = qkv.tile([P, N_ST, D], bf16)

            attn_local = []
            attn_all = []
            for qc in range(N_SC):
                al = dram.tile([OC, SC], bf16, name=f"attn_local{qc}")
                ag = dram.tile([NQ * D, SC], bf16, name=f"attn_all{qc}",
                               addr_space="Shared")
                attn_local.append(al)
                attn_all.append(ag)

            # ---- interleaved: proj(sc) -> attention(ready qc) -> gather ----
            # attention for qc can only run once every K/V chunk it reads has
            # been projected; for a causal mask need[qc] == qc, for a dense
            # mask need[qc] == N_SC-1 (attention all trails the projections).
            need = {}
            for qc in range(N_SC):
                kts_q = [kt for kt in range(N_ST) if binfo[(kt, qc)] != 'skip']
                need[qc] = max(kt // (SC // P) for kt in kts_q) if kts_q else 0
            for sc in range(N_SC):
                ssl = slice(sc * SC, (sc + 1) * SC)
                if sc == 0:
                    x_t = x_t0
                else:
                    x_t = xs.tile([P, N_HT, SC], bf16, tag="x_t")
                    nc.sync.dma_start(x_t[:], x4.ap()[sc])
                # Qt (4 head-tiles) and Kt (1 tile), with RoPE
                for ot in range(QH + 1):
                    ps = ps_s.tile([P, SC], f32, tag="ps")
                    for h in range(N_HT):
                        nc.tensor.matmul(
                            ps[:], w_sb[:, ot, h, :], x_t[:, h, :],
                            start=(h == 0), stop=(h == N_HT - 1))
                    pre = tmp.tile([P, SC], bf16, tag="pre")
                    nc.vector.tensor_copy(pre[:], ps[:])
                    psr = ps_r.tile([P, SC], f32)
                    nc.tensor.matmul(psr[:], rot_sb[:], pre[:],
                                     start=True, stop=True)
                    t1 = tmp.tile([P, SC], f32, tag="t1")
                    nc.vector.tensor_tensor(t1[:], pre[:], cos_sb[:, ssl], ALU.mult)
                    t2 = tmp.tile([P, SC], f32, tag="t2")
                    nc.vector.tensor_tensor(t2[:], psr[:], sin_sb[:, ssl], ALU.mult)
                    dst = Q_all[:, ot, ssl] if ot < QH else K_all[:, ssl]
                    nc.vector.tensor_tensor(dst, t1[:], t2[:], ALU.add)
                # V in [S, D] layout
                for st in range(SC // P):
                    psv_full = ps_r.tile([P, SC], f32, tag="psr", name="psv")
                    psv = psv_full[:, :D]
                    for h in range(N_HT):
                        nc.tensor.matmul(
                            psv[:], x_t[:, h, st * P:(st + 1) * P],
                            w_sb[:, 5, h, :],
                            start=(h == 0), stop=(h == N_HT - 1))
                    nc.vector.tensor_copy(V_all[:, sc * (SC // P) + st, :], psv[:])

                # ---- attention for every qc whose K/V chunks are now ready --
                # one-head software pipeline: emit head h+1's scores before
                # head h's PV so the PE has work while ScalarE runs the exps.
                ready = [q for q in range(N_SC) if need[q] == sc]

                def emit_pv(head, kts, exp_tiles, qc):
                    psa = ps_a.tile([P, SC], f32, tag="pv", name="psa")
                    pso = ps_a.tile([P, SC], f32, tag="ones", name="pso")
                    for i, kt in enumerate(kts):
                        ex, lo = exp_tiles[kt]
                        n = SC - lo
                        nc.tensor.matmul(psa[:, lo:], V_all[:, kt, :], ex[:, :n],
                                         start=(i == 0), stop=(i == len(kts) - 1))
                        nc.tensor.matmul(pso[:, lo:], ones_sb[:], ex[:, :n],
                                         start=(i == 0), stop=(i == len(kts) - 1))
                    # copy PV out of PSUM quickly so the bank frees; divide later
                    acop = att.tile([P, SC], f32, tag="acop", name="acop")
                    nc.vector.tensor_copy(acop[:], psa[:])
                    rec = tmp.tile([P, SC], f32, tag="rec", name="rec")
                    nc.vector.reciprocal(rec[:], pso[:])
                    a_sb = att.tile([P, SC], bf16, tag="a_sb", name="a_sb")
                    nc.vector.tensor_tensor(a_sb[:], acop[:], rec[:], ALU.mult)
                    nc.sync.dma_start(
                        attn_local[qc][head * P:(head + 1) * P, :], a_sb[:])

                for qc in ready:
                    qsl = slice(qc * SC, (qc + 1) * SC)
                    # deeper software pipeline where exp-tile liveness allows:
                    # (heads in flight) * (tiles/head) must stay under the
                    # expp pool depth (27)
                    n_kts = len([kt for kt in range(N_ST)
                                 if binfo[(kt, qc)] != 'skip'])
                    depth = max(1, min(QH, 24 // max(n_kts, 1)))
                    pend = []
                    for head in range(QH):
                        kts = [kt for kt in range(N_ST)
                               if binfo[(kt, qc)] != 'skip']
                        exp_tiles = {}
                        for kt in kts:
                            cls = binfo[(kt, qc)]
                            lo = 0 if cls == 'plain' else cls[1]
                            n = SC - lo
                            pss = ps_s.tile([P, SC], f32, tag="ps")
                            nc.tensor.matmul(
                                pss[:, :n], K_all[:, kt * P:(kt + 1) * P],
                                Q_all[:, head, qc * SC + lo:(qc + 1) * SC],
                                start=True, stop=True)
                            ex = expp.tile([P, SC], bf16, tag="ex")
                            nc.scalar.activation(ex[:, :n], pss[:, :n], AF.Exp,
                                                 scale=float(SCALING))
                            if cls != 'plain':
                                nc.vector.tensor_tensor(
                                    ex[:, :n], ex[:, :n],
                                    mask_sb[:, cls[0], lo:], ALU.mult)
                            exp_tiles[kt] = (ex, lo)
                        pend.append((head, kts, exp_tiles, qc))
                        if len(pend) >= depth:
                            emit_pv(*pend.pop(0))
                    while pend:
                        emit_pv(*pend.pop(0))
                    nc.gpsimd.collective_compute(
                        "AllGather", mybir.AluOpType.bypass,
                        replica_groups=[list(range(NCORES))],
                        ins=[attn_local[qc].opt()], outs=[attn_all[qc].opt()])

            # ---------------- o_proj slice, per gathered chunk ----------------
            wo_sb = weights.tile([P, NQ * D // P, OC], bf16, tag="wbig")
            nc.sync.dma_start(wo_sb[:], wo3.ap())
            n_at = NQ * D // P          # 32 feature tiles
            for qc in range(N_SC):
                a_t = xs.tile([P, n_at, SC], bf16, tag="x_t")
                src_r = attn_all[qc].rearrange("(o p) s -> p o s", p=P)
                for st2 in range(SC // P):
                    nc.sync.dma_start(
                        a_t[:, :, st2 * P:(st2 + 1) * P],
                        src_r[:, :, st2 * P:(st2 + 1) * P])
                for st2 in range(SC // P):
                    pso2 = ps_s.tile([P, SC], f32, tag="ps", name="pso2")
                    for kt in range(n_at):
                        nc.tensor.matmul(
                            pso2[:], a_t[:, kt, st2 * P:(st2 + 1) * P],
                            wo_sb[:, kt, :],
                            start=(kt == 0), stop=(kt == n_at - 1))
                    o_sb = outp.tile([P, OC], f32)
                    nc.vector.tensor_copy(o_sb[:], pso2[:])
                    nc.sync.dma_start(
                        out.ap()[(qc * (SC // P) + st2) * P:
                                 (qc * (SC // P) + st2 + 1) * P, :], o_sb[:])

    nc.compile()
    return nc


def _prep_inputs(hidden_states, cos, sin, attention_mask, wq, wk, wv, wo,
                 mask_blocks):
    hs = hidden_states[0]  # [S, H] f32
    # x4[sc, p, o, j] = hs[sc*512+j, o*128+p]
    x4 = np.ascontiguousarray(
        hs.reshape(N_SC, SC, N_HT, P).transpose(0, 3, 2, 1)).astype(BF16)
    cosT = np.ascontiguousarray(cos[0].T).astype(np.float32)
    sinT = np.ascontiguousarray(sin[0].T).astype(np.float32)
    # rot(q)[m] = -q[m+64] for m<64 ; +q[m-64] for m>=64.
    # matmul computes out[m] = sum_k lhsT[k, m] q[k]
    rot = np.zeros((D, D), np.float32)
    half = D // 2
    for m in range(half):
        rot[m + half, m] = -1.0
    for m in range(half, D):
        rot[m - half, m] = 1.0
    rotP = rot.astype(BF16)
    if mask_blocks:
        maskm = np.concatenate(mask_blocks, axis=1).astype(BF16)
        maskm = np.ascontiguousarray(maskm)
    in_maps = []
    for c in range(NCORES):
        wq_s = wq[c * OC:(c + 1) * OC, :]
        wk_s = wk[c * KVC:(c + 1) * KVC, :]
        wv_s = wv[c * KVC:(c + 1) * KVC, :]
        # wqkv_cat.T: [H, 768]; w6[g, p, o, j] = wqkv_cat_T[o*128+p, g*128+j]
        wcat = np.concatenate([wq_s, wk_s, wv_s], axis=0).T  # [H, 768]
        w6 = np.ascontiguousarray(
            wcat.reshape(N_HT, P, 6, D).transpose(2, 1, 0, 3)).astype(BF16)
        # wo3[p, o, j] = wo_t[o*128+p, j],  wo_t = wo[cOC:(c+1)OC, :].T [4096, 512]
        wo_c = wo[c * OC:(c + 1) * OC, :].T  # [4096, 512]
        wo3 = np.ascontiguousarray(
            wo_c.reshape(NQ * D // P, P, OC).transpose(1, 0, 2)).astype(BF16)
        m = {"x4": x4, "w6": w6, "wo3": wo3, "cosT": cosT,
             "sinT": sinT, "rotP": rotP}
        if mask_blocks:
            m["maskm"] = maskm
        in_maps.append(m)
    return in_maps


def kernel(hidden_states, cos, sin, attention_mask, wq, wk, wv, wo):
    global LAST_RESULT
    from concourse.bass_utils import run_bass_kernel_spmd

    hidden_states = np.asarray(hidden_states, dtype=np.float32)
    cos = np.asarray(cos, dtype=np.float32)
    sin = np.asarray(sin, dtype=np.float32)
    attention_mask = np.asarray(attention_mask, dtype=np.float32)
    wq = np.asarray(wq, dtype=np.float32)
    wk = np.asarray(wk, dtype=np.float32)
    wv = np.asarray(wv, dtype=np.float32)
    wo = np.asarray(wo, dtype=np.float32)

    binfo, mask_blocks = _classify_mask(attention_mask[0, 0])
    key = tuple(sorted(binfo.items())) + (len(mask_blocks),)
    if key not in _CACHE:
        _CACHE[key] = _build(binfo, len(mask_blocks))
    nc = _CACHE[key]

    in_maps = _prep_inputs(hidden_states, cos, sin, attention_mask,
                           wq, wk, wv, wo, mask_blocks)
    res = run_bass_kernel_spmd(nc, in_maps, core_ids=list(range(NCORES)),
                               trace=TRACE)
    LAST_RESULT = res
    out_full = np.empty((S, NQ * D), np.float32)
    for c in range(NCORES):
        out_full[:, c * OC:(c + 1) * OC] = res.results[c]["out"]
    return out_full.reshape(B, S, NQ * D)
